# revision 23
# baseline (speedup 1.0000x reference)
"""3-layer GCN forward on 8 TRN2 NeuronCores (Bass/Tile), v2.

Math: per layer, out = dinv * ((A+I) @ T) @ W + b with T = dinv*h,
dinv = 1/sqrt(deg+1); leaky_relu(0.2) between layers. Self-loops are
ordinary tokens (node gathers its own T row), so there is no separate
+T path.

Device plan (dst-sharded, one SPMD program on 8 cores):
- Feature table = pair-packed DRAM [NPAIR, 128] bf16: pair row p holds
  nodes 2p and 2p+1 (64 feats each). All pair rows fit int16, so one
  dma_gather base covers the whole graph (no A/B halves). Layer 0's
  table is a host-staged input (x prescaled by dinv, bf16); later
  tables are the AllGather outputs themselves - no copies, no zeroing.
- Tokens (edges + self-loops) are grouped by dst tile (128 nodes) and
  by src parity (which 64-col half of the gathered pair row), padded
  to 128-token planes shared across cores.
- Segment sums on PE: per plane, matmul(lhsT=msg[:,pl,q*64:+64],
  rhs=onehot[128,128]) accumulates into a PSUM window [64, 128] per
  dst tile (start/stop over the tile's planes). Onehot built on DVE:
  is_equal(iota_row, sid) where sid[p, pl] = dst slot of token, -1 pads.
- Epilogue per tile: psum_h = matmul(lhsT=stage[:, tile], rhs=W);
  x dinv, +b, leaky, x dinv -> bf16 cc_in (node-major [BP,64], whose
  bytes are exactly the pair view [BP/2, 128]); AllGather -> next table.
- Host: balanced node->tile permutation (equalizes tokens per
  (tile,parity) so cross-core shared plane counts stay tight); output
  unpermuted on host.
"""
import numpy as np

NEG_SLOPE = 0.2
_TIMING = False  # strip custom-DMA sems so TimelineSim can run
_DEBUG = False


class _Cfg:
    def __init__(self, n_nodes, d_in=64, d_out=4, sup_tiles=7):
        self.W = 8
        self.N = n_nodes
        self.D = d_in
        self.DOUT = d_out
        self.BLK = (n_nodes + self.W - 1) // self.W
        self.BP = ((self.BLK + 127) // 128) * 128
        self.NT = self.W * self.BP
        self.NPAIR = self.NT // 2
        self.NTILES = self.BP // 128
        # super-tile size (tiles per gather/psum chunk)
        self.SUP = sup_tiles if self.NTILES % sup_tiles == 0 else 1
        self.NSUP = self.NTILES // self.SUP
        assert self.NPAIR <= 32768, self.NPAIR


def _balance_tiles(cfg, tok_counts):
    """Assign this core's nodes (local ids) to (tile, slot) so that
    per-(tile,parity) token counts are even. tok_counts: [nloc, 2] int.
    Returns perm[nloc] -> tile*128+slot."""
    nloc = tok_counts.shape[0]
    ntiles = cfg.NTILES
    cap = np.full(ntiles, 128, np.int64)
    # leave fake slots distributed: capacity 128 each, total >= nloc
    load = np.zeros((ntiles, 2), np.float64)
    order = np.argsort(-(tok_counts.sum(1)))
    perm = np.zeros(nloc, np.int64)
    slots_used = np.zeros(ntiles, np.int64)
    for n in order:
        t0, t1 = tok_counts[n]
        # pick open tile minimizing resulting max-parity load
        best, bestv = -1, None
        cand = np.where(slots_used < cap)[0]
        v = np.maximum(load[cand, 0] + t0, load[cand, 1] + t1)
        best = cand[np.argmin(v)]
        perm[n] = best * 128 + slots_used[best]
        slots_used[best] += 1
        load[best, 0] += t0
        load[best, 1] += t1
    return perm


def _preprocess(cfg, edge_index):
    """Build shared plane schedule + per-core gidx/sid + per-core node
    permutations and dinv."""
    W, N, BLK, BP = cfg.W, cfg.N, cfg.BLK, cfg.BP
    src = np.asarray(edge_index[0], np.int64)
    dst = np.asarray(edge_index[1], np.int64)
    deg = np.bincount(dst, minlength=N).astype(np.float64) + 1.0
    dinv = (1.0 / np.sqrt(deg)).astype(np.float32)

    # per-core local node -> global row permutation
    perms = []        # per core: local node i -> row offset within block
    core_edges = []   # per core: (src_global, dst_local)
    for c in range(W):
        lo, hi = c * BLK, min((c + 1) * BLK, N)
        m = (dst >= lo) & (dst < hi)
        s_c, d_c = src[m], dst[m] - lo
        core_edges.append((s_c, d_c))
        nloc = hi - lo
        # token counts per (node, parity-of-src-row); self token too
        # NOTE: parity depends on src global ROW, which depends on the
        # src core's permutation -> chicken & egg. Use src node id
        # parity as proxy for balancing only (exact counts computed
        # later once all perms are fixed; parity imbalance is tiny).
        tc = np.zeros((nloc, 2), np.int64)
        np.add.at(tc, (d_c, s_c % 2), 1)
        own = np.arange(nloc)
        tc[own, own % 2] += 1  # self token (proxy parity)
        perms.append(_balance_tiles(cfg, tc))

    # global row of node n
    grow = np.zeros(N, np.int64)
    for c in range(W):
        lo, hi = c * BLK, min((c + 1) * BLK, N)
        grow[lo:hi] = c * BP + perms[c]

    # build token lists per core: (pair, parity, tile, slot)
    # shared plane counts: planes[t][q] = max over cores
    per_core_tok = []
    for c in range(W):
        lo, hi = c * BLK, min((c + 1) * BLK, N)
        s_c, d_c = core_edges[c]
        own = np.arange(hi - lo)
        s_all = np.concatenate([s_c, own + lo])      # self tokens
        d_all = np.concatenate([d_c, own])
        r = grow[s_all]
        pair, par = r >> 1, r & 1
        pos = perms[c][d_all]
        tile, slot = pos >> 7, pos & 127
        per_core_tok.append((pair, par, tile, slot))

    ntiles = cfg.NTILES
    counts = np.zeros((W, ntiles, 2), np.int64)
    for c in range(W):
        _, par, tile, _ = per_core_tok[c]
        np.add.at(counts[c], (tile, par), 1)
    planes_tq = (counts.max(0) + 127) // 128  # [ntiles, 2]
    planes_tq = np.maximum(planes_tq, 1)

    # canonical plane order: for g in sup, for q in (0,1),
    #   for t in tiles(g), planes(t,q)
    plane_tile = []   # per plane: (tile, q)
    chunk_meta = []   # per chunk: (plane0, nplanes, g, q)
    for g in range(cfg.NSUP):
        tl = range(g * cfg.SUP, (g + 1) * cfg.SUP)
        for q in (0, 1):
            p0 = len(plane_tile)
            for t in tl:
                for _ in range(planes_tq[t, q]):
                    plane_tile.append((t, q))
            chunk_meta.append((p0, len(plane_tile) - p0, g, q))
    nplanes = len(plane_tile)
    ntok = nplanes * 128

    # per-core gidx/sid fill
    gidxs, sids = [], []
    for c in range(W):
        pair, par, tile, slot = per_core_tok[c]
        gi = np.zeros(ntok, np.int64)          # pad -> pair row 0
        sd = np.full(ntok, -1.0, np.float32)   # pad -> no slot
        # bucket tokens by (tile, q), fill planes in canonical order
        plane_base = {}
        off = 0
        for pl, (t, q) in enumerate(plane_tile):
            plane_base.setdefault((t, q), []).append(pl)
        key = tile * 2 + par
        order = np.argsort(key, kind='stable')
        ks, ps, ss = key[order], pair[order], slot[order]
        bounds = np.searchsorted(ks, np.arange(ntiles * 2 + 1))
        for t in range(ntiles):
            for q in (0, 1):
                a, b = bounds[t * 2 + q], bounds[t * 2 + q + 1]
                cnt = b - a
                pls = plane_base[(t, q)]
                assert cnt <= len(pls) * 128, (c, t, q, cnt)
                for j, pl in enumerate(pls):
                    u, v = a + j * 128, min(a + (j + 1) * 128, b)
                    if u >= v:
                        break
                    base = pl * 128
                    gi[base:base + (v - u)] = ps[u:v]
                    sd[base:base + (v - u)] = ss[u:v]
        gidxs.append(gi.astype(np.int16))
        sids.append(sd)
    sched = dict(nplanes=nplanes, ntok=ntok, plane_tile=plane_tile,
                 chunks=chunk_meta, planes_tq=planes_tq)
    return dinv, perms, sched, gidxs, sids


def _wrap16(a):
    a = np.asarray(a, np.int16)
    assert a.size % 16 == 0
    w = np.ascontiguousarray(a.reshape(-1, 16).T)
    return np.tile(w, (8, 1))


def _build(cfg, sched):
    import concourse.bacc as bacc
    import concourse.mybir as mybir
    import concourse.tile as tile
    import concourse.masks as masks

    D, DOUT = cfg.D, cfg.DOUT
    BP, NPAIR, NTILES = cfg.BP, cfg.NPAIR, cfg.NTILES
    SUP, NSUP = cfg.SUP, cfg.NSUP
    SUPN = SUP * 128  # nodes per super-tile
    f32, bf16, i16 = mybir.dt.float32, mybir.dt.bfloat16, mybir.dt.int16
    EQ = mybir.AluOpType.is_equal
    nplanes = sched['nplanes']
    plane_tile = sched['plane_tile']
    chunks = sched['chunks']

    nc = bacc.Bacc(None, target_bir_lowering=False)
    x_table = nc.dram_tensor("x_table", [NPAIR, 128], bf16,
                             kind="ExternalInput")
    w0 = nc.dram_tensor("w0", [D, D], f32, kind="ExternalInput")
    w1 = nc.dram_tensor("w1", [D, D], f32, kind="ExternalInput")
    w2 = nc.dram_tensor("w2", [D, DOUT], f32, kind="ExternalInput")
    b01 = nc.dram_tensor("b01", [128, 2 * D], f32, kind="ExternalInput")
    b2b = nc.dram_tensor("b2b", [128, DOUT], f32, kind="ExternalInput")
    dinv_in = nc.dram_tensor("dinv_blk", [128, NTILES], f32,
                             kind="ExternalInput")
    iota_in = nc.dram_tensor("iota_rep", [128, 128], bf16,
                             kind="ExternalInput")
    gidx_in = nc.dram_tensor("gidx", [128, sched['ntok'] // 16], i16,
                             kind="ExternalInput")
    sid_in = nc.dram_tensor("sid", [128, nplanes], bf16,
                            kind="ExternalInput")
    outr = nc.dram_tensor("outr", [BP, DOUT], f32, kind="ExternalOutput")

    cc_in = nc.dram_tensor("cc_in", [BP, D], bf16)
    cc_out = [nc.dram_tensor(f"cc_out{i}", [NPAIR, 128], bf16)
              for i in (0, 1)]
    dbg = [nc.dram_tensor(f"dbg{i}", [NPAIR, 128], bf16,
                          kind="ExternalOutput") for i in (0, 1)] \
        if _DEBUG else None
    dbg_st = nc.dram_tensor("dbg_st", [128, NTILES * D], f32,
                            kind="ExternalOutput") if _DEBUG else None
    np0 = chunks[0][1]
    dbg_oh = nc.dram_tensor("dbg_oh", [128, np0 * 128], bf16,
                            kind="ExternalOutput") if _DEBUG else None
    dbg_msg = nc.dram_tensor("dbg_msg", [128, np0 * 128], bf16,
                             kind="ExternalOutput") if _DEBUG else None

    with tile.TileContext(nc) as tc:
        with (
            tc.tile_pool(name="const", bufs=1) as cpool,
            tc.tile_pool(name="msg", bufs=3) as msgpool,
            tc.tile_pool(name="oh", bufs=3) as ohpool,
            tc.tile_pool(name="stage", bufs=2) as stpool,
            tc.tile_pool(name="epi", bufs=3) as epi,
            tc.tile_pool(name="psum", bufs=2, space="PSUM") as psum,
            tc.tile_pool(name="psum_e", bufs=2, space="PSUM") as psum_e,
        ):
            gsem = [nc.alloc_semaphore(f"gsem{i}") for i in range(4)]
            gcnt = [0] * 4
            cc_sem = nc.alloc_semaphore("cc_sem")
            cc_cnt = [0]

            w0t = cpool.tile([D, D], f32)
            w1t = cpool.tile([D, D], f32)
            w2t = cpool.tile([D, DOUT], f32)
            nc.sync.dma_start(w0t[:], w0[:])
            nc.sync.dma_start(w1t[:], w1[:])
            nc.sync.dma_start(w2t[:], w2[:])
            b01t = cpool.tile([128, 2 * D], f32)
            nc.sync.dma_start(b01t[:], b01[:])
            b2t = cpool.tile([128, DOUT], f32)
            nc.sync.dma_start(b2t[:], b2b[:])
            dinvt = cpool.tile([128, NTILES], f32)
            nc.sync.dma_start(dinvt[:], dinv_in[:])
            gixt = cpool.tile([128, sched['ntok'] // 16], i16)
            nc.sync.dma_start(gixt[:], gidx_in[:])
            sidt = cpool.tile([128, nplanes], bf16)
            nc.sync.dma_start(sidt[:], sid_in[:])
            iot = cpool.tile([128, 128], bf16)
            nc.sync.dma_start(iot[:], iota_in[:])
            ident = cpool.tile([128, 128], f32)
            masks.make_identity(nc, ident[:])

            from concourse.bass import AP

            for layer in range(3):
                table = x_table if layer == 0 else cc_out[layer - 1]
                wt = (w0t, w1t, w2t)[layer]
                stage = stpool.tile([128, NTILES * D], f32, tag="stage")

                for g in range(NSUP):
                    # two parity chunks for this super-tile
                    cms, ohs, thr, sems = [], [], [], []
                    for q in (0, 1):
                        p0, np_, gg, qq = chunks[g * 2 + q]
                        assert (gg, qq) == (g, q)
                        msg = msgpool.tile([128, np_ * 128], bf16,
                                           tag="msg")
                        k = (layer * 2 * NSUP + g * 2 + q) % 4
                        gx = gixt[:, p0 * 8:(p0 + np_) * 8]
                        gth = nc.gpsimd.dma_gather(
                            msg[:].rearrange("p (a c) -> p a c", c=128),
                            table[:, :], gx, np_ * 128,
                            np_ * 128, 128, single_packet=False)
                        if not _TIMING:
                            gth.then_inc(gsem[k], 16)
                        gcnt[k] += 16
                        oh = ohpool.tile([128, np_ * 128], bf16, tag="oh")
                        ia = iot[:]
                        iv = AP(ia.tensor, ia.offset,
                                [list(ia.ap[0]), [0, np_],
                                 list(ia.ap[1])])
                        sa = sidt[:, p0:p0 + np_]
                        sv = AP(sa.tensor, sa.offset,
                                [list(sa.ap[0]), list(sa.ap[1]),
                                 [0, 128]])
                        nc.vector.tensor_tensor(
                            oh[:].rearrange("p (a c) -> p a c", c=128),
                            iv, sv, EQ)
                        if _DEBUG and layer == 0 and g == 0 and q == 0:
                            dmc = nc.sync.dma_start(dbg_msg[:], msg[:])
                            if not _TIMING:
                                dmc._wait_ge(gsem[k], gcnt[k])
                            nc.sync.dma_start(dbg_oh[:], oh[:])
                        cms.append((p0, np_, msg))
                        ohs.append(oh)
                        thr.append(gcnt[k])
                        sems.append(gsem[k])

                    pg = psum.tile([128, 512], f32, tag="pg")
                    pls = []
                    for trel in range(SUP):
                        t = g * SUP + trel
                        for q in (0, 1):
                            p0, np_, _ = cms[q]
                            for j in range(np_):
                                if plane_tile[p0 + j][0] == t:
                                    pls.append((trel, q, j))
                    for i, (trel, q, j) in enumerate(pls):
                        _, _, msg = cms[q]
                        mm = nc.tensor.matmul(
                            pg[:, trel * D:(trel + 1) * D],
                            ohs[q][:, j * 128:(j + 1) * 128],
                            msg[:, j * 128 + q * D:
                                j * 128 + q * D + D],
                            start=(i == 0), stop=(i == len(pls) - 1))
                        if not _TIMING:
                            mm._wait_ge(sems[q], thr[q])
                    nc.vector.tensor_copy(
                        stage[:, g * SUP * D:(g + 1) * SUP * D],
                        pg[:, 0:SUP * D])

                if _DEBUG and layer == 2:
                    nc.sync.dma_start(dbg_st[:, 0:NTILES * D],
                                      stage[:, 0:NTILES * D])
                # epilogue per super-tile
                for g in range(NSUP):
                    if layer < 2:
                        ob = epi.tile([128, SUP, D], bf16, tag="ob")
                    else:
                        ob = epi.tile([128, SUP, DOUT], f32, tag="ob2")
                    for trel in range(SUP):
                        t = g * SUP + trel
                        ztp = psum_e.tile([128, 512], f32, tag="ztp")
                        nc.tensor.transpose(
                            ztp[0:D, 0:128], stage[:, t * D:(t + 1) * D],
                            ident[:])
                        zts = epi.tile([D, 128], f32, tag="zts")
                        nc.vector.tensor_copy(zts[:], ztp[0:D, 0:128])
                        if layer < 2:
                            ph = psum_e.tile([128, 512], f32, tag="ph")
                            nc.tensor.matmul(
                                ph[:, 0:D], zts[:],
                                wt[:], start=True, stop=True)
                            h = epi.tile([128, D], f32, tag="h")
                            nc.vector.tensor_scalar_mul(
                                h[:], ph[:, 0:D], dinvt[:, t:t + 1])
                            nc.vector.tensor_add(
                                h[:], h[:],
                                b01t[:, layer * D:(layer + 1) * D])
                            hl = epi.tile([128, D], f32, tag="hl")
                            nc.scalar.mul(hl[:], h[:], NEG_SLOPE)
                            nc.vector.tensor_max(hl[:], hl[:], h[:])
                            nc.vector.tensor_scalar_mul(
                                ob[:, trel, :], hl[:], dinvt[:, t:t + 1])
                        else:
                            ph = psum_e.tile([128, 512], f32, tag="ph")
                            nc.tensor.matmul(
                                ph[:, 0:DOUT], zts[:],
                                wt[:], start=True, stop=True)
                            h = epi.tile([128, DOUT], f32, tag="h2")
                            nc.vector.tensor_scalar_mul(
                                h[:], ph[:, 0:DOUT], dinvt[:, t:t + 1])
                            nc.vector.tensor_add(ob[:, trel, :], h[:],
                                                 b2t[:])
                    dst = cc_in if layer < 2 else outr
                    nc.sync.dma_start(
                        dst[g * SUPN:(g + 1) * SUPN, :].rearrange(
                            "(a p) c -> p a c", p=128), ob[:])

                if layer < 2:
                    with tc.tile_critical():
                        cci = nc.gpsimd.collective_compute(
                            "AllGather", mybir.AluOpType.bypass,
                            ins=[cc_in[:, :].rearrange(
                                "(p two) c -> p (two c)", two=2)],
                            outs=[cc_out[layer][:, :]],
                            replica_groups=[list(range(cfg.W))])
                        cci.then_inc(cc_sem, 1)
                        cc_cnt[0] += 1
                        nc.gpsimd.wait_ge(cc_sem, cc_cnt[0])
                    if _DEBUG:
                        nc.sync.dma_start(dbg[layer][:],
                                          cc_out[layer][:])
    nc.compile()
    return nc


_CACHE = {}


def _get_program(key, cfg, edge_index):
    if key in _CACHE:
        return _CACHE[key]
    dinv, perms, sched, gidxs, sids = _preprocess(cfg, edge_index)
    nc = _build(cfg, sched)
    _CACHE[key] = (nc, dinv, perms, sched, gidxs, sids)
    return _CACHE[key]


def kernel(x, edge_index, W0, b0, W1, b1, W2, b2, _cfg=None, _sim=False):
    import ml_dtypes
    x = np.asarray(x, np.float32)
    edge_index = np.asarray(edge_index)
    N, D = x.shape
    DOUT = np.asarray(W2).shape[1]
    cfg = _cfg or _Cfg(N, D, DOUT)
    nc, dinv, perms, sched, gidxs, sids = _get_program(
        (N, edge_index.shape[1]), cfg, edge_index)

    BP, BLK, Wc, NTILES = cfg.BP, cfg.BLK, cfg.W, cfg.NTILES

    xs = (x * dinv[:, None]).astype(ml_dtypes.bfloat16)
    b01 = np.zeros((128, 2 * D), np.float32)
    b01[:, :D] = np.asarray(b0, np.float32)[None, :]
    b01[:, D:] = np.asarray(b1, np.float32)[None, :]
    b2t = np.tile(np.asarray(b2, np.float32)[None, :], (128, 1))
    iota = np.ascontiguousarray(
        np.tile(np.arange(128, dtype=np.float32)[None, :],
                (128, 1))).astype(ml_dtypes.bfloat16)

    in_maps = []
    for c in range(Wc):
        lo, hi = c * BLK, min((c + 1) * BLK, N)
        db = np.zeros(BP, np.float32)
        db[perms[c]] = dinv[lo:hi]
        dinv_blk = np.ascontiguousarray(
            db.reshape(NTILES, 128).T).astype(np.float32)
        sid = sids[c].reshape(sched['nplanes'], 128).T
        im = dict(
            w0=np.asarray(W0, np.float32), w1=np.asarray(W1, np.float32),
            w2=np.asarray(W2, np.float32), b01=b01, b2b=b2t,
            dinv_blk=dinv_blk, iota_rep=iota,
            gidx=_wrap16(gidxs[c]),
            sid=np.ascontiguousarray(sid).astype(ml_dtypes.bfloat16),
        )
        in_maps.append(im)

    # x_table is global (all cores see all blocks): assemble full table
    xt_full = np.zeros((cfg.NT, D), ml_dtypes.bfloat16)
    for c in range(Wc):
        lo, hi = c * BLK, min((c + 1) * BLK, N)
        xt_full[c * BP + perms[c]] = xs[lo:hi]
    xt_full = np.ascontiguousarray(xt_full.reshape(cfg.NPAIR, 128))
    for c in range(Wc):
        in_maps[c]["x_table"] = xt_full

    if _sim:
        from concourse import bass_interp
        sim = bass_interp.MultiCoreSim(nc, Wc)
        for c in range(Wc):
            for k, v in in_maps[c].items():
                sim.cores[c].tensor(k)[:] = v
            sim.cores[c].mem_tensor("outr")[:] = 0
        sim.simulate()
        results = [np.array(sim.cores[c].mem_tensor("outr")).reshape(BP, DOUT)
                   for c in range(Wc)]
    else:
        from concourse.bass_utils import run_bass_kernel_spmd
        res = run_bass_kernel_spmd(nc, in_maps, list(range(Wc)))
        results = [res.results[c]["outr"] for c in range(Wc)]

    out = np.zeros((N, DOUT), np.float32)
    for c in range(Wc):
        lo, hi = c * BLK, min((c + 1) * BLK, N)
        out[lo:hi] = results[c][perms[c]]
    return out
